# revision 30
# baseline (speedup 1.0000x reference)
"""BitNet transformer layer (B=1, S=2048, H=2560, NH=20, NKV=5, HD=128, FF=6912)
on 8 Trainium2 NeuronCores.

Sharding: sequence-interleaved data parallel. Core c owns tokens {8*i + c}.
All weights are replicated (ternary-quantized on host to exact {-1,0,+1} fp8,
so every projection matmul is integer-exact with fp32 PSUM accumulation).

v2 design notes:
- fp32(r) attention path end to end (q/k/v/probs) - fp32r matmuls run at
  bf16 rate for moving dims >= 256, and the extra precision keeps the
  int8-quantizer rounding cliffs downstream from amplifying bf16 noise.
- Q/K projections run operand-swapped (weights stationary) so q^T/k^T come
  out of the PE feature-major directly; RoPE is applied feature-major with
  the half-rotation materialized by two PSUM->SBUF DMAs.
- Per-kv-head AllGather (5 collectives) so attention can start as soon as
  the first head's K/V has arrived; each AG overlaps the Q projection.
- o and m (down-proj input) are normalized+quantized feature-major with
  per-column scales (broadcast via tiny PE outer products), eliminating
  all o/m quant transposes and the softmax-denominator transposes.
- gate/up run operand-swapped producing feature-major m directly.
"""

import sys

import numpy as np

if "/opt/trn_rl_repo" not in sys.path:
    sys.path.insert(0, "/opt/trn_rl_repo")

import ml_dtypes

import concourse.bass as bass
import concourse.tile as tile
from concourse import bacc, mybir
from concourse import bass_utils
from concourse import bass_isa

F32 = mybir.dt.float32
F32R = mybir.dt.float32r
BF16 = mybir.dt.bfloat16
FP8 = mybir.dt.float8e4
AF = mybir.ActivationFunctionType
ALU = mybir.AluOpType

NCORES = 8
S, H, NH, NKV, HD, FF = 2048, 2560, 20, 5, 128, 6912
T = S // NCORES            # 256 tokens per core
P = 128
TP = T // P                # 2 token ptiles per core
HB = H // P                # 20 hidden blocks
FB = FF // P               # 54 ff blocks
GQ = NH // NKV             # 4 q heads per kv head
KV = NKV * HD              # 640
EPS = 1e-5
MAGIC = 12582912.0         # 1.5*2^23: (x+MAGIC)-MAGIC == rne-round(x) for |x|<2^22
NEG = -1e30
HGRP = 10                  # contraction blocks per weight macro-DMA
H64 = HD // 2
AGC = HD * T + T * HD      # elems per core per AG section (K part + V part)


def _bcast_dma(nc, out_tile, dram_ap, offset_elems, n):
    """DMA a [n] f32 DRAM vector to [128, n] SBUF, broadcast over partitions."""
    src = bass.AP(tensor=dram_ap.tensor, offset=offset_elems, ap=[[0, P], [1, n]])
    nc.gpsimd.dma_start(out=out_tile, in_=src)


def _build_nc():
    nc = bacc.Bacc("TRN2", target_bir_lowering=False, debug=False,
                   num_devices=NCORES)

    aps = {}
    def inp(name, shape, dt):
        aps[name] = nc.dram_tensor(name, shape, dt, kind="ExternalInput").ap()
    inp("x", [T, H], F32)
    inp("cosT", [HD, T], F32)     # feature-major rope tables
    inp("sinrT", [HD, T], F32)    # sign-folded: rows 0..63 negated
    inp("mask", [NCORES, P, P], BF16)
    inp("wq", [H, H], FP8)
    inp("wk", [H, KV], FP8)
    inp("wv", [H, KV], FP8)
    inp("wo", [H, H], FP8)
    inp("wg", [H, FF], FP8)
    inp("wu", [H, FF], FP8)
    inp("wd", [FF, H], FP8)
    inp("wln", [H], F32)
    inp("wsub_fm", [P, HB], F32)   # w_attn_sub feature-major [d, block]
    inp("wpost", [H], F32)
    inp("wffn_fm", [P, FB], F32)   # w_ffn_sub feature-major [d, block]
    inp("wsc", [5], F32)   # [wsq, wsk, wsv, wso, wsd]
    aps["out"] = nc.dram_tensor("out", [T, H], F32, kind="ExternalOutput").ap()

    with tile.TileContext(nc) as tc:
        _emit(nc, tc, aps)

    nc.compile()
    return nc


def _nq_stats_chunk(nc, work, src_slice, w_b_slice, sqp_col, mxp_col, use_gps):
    """Square-accum + |x*w| max for one [128, nn] chunk."""
    nn = src_slice.shape[-1]
    scr = work.tile([P, 512], F32, tag="c512a", name="c512a")[:, :nn]
    nc.scalar.activation(out=scr, in_=src_slice, func=AF.Square,
                         accum_out=sqp_col)
    xw = work.tile([P, 512], F32, tag="c512b", name="c512b")[:, :nn]
    eng = nc.gpsimd if use_gps else nc.vector
    eng.tensor_tensor(out=xw, in0=src_slice, in1=w_b_slice, op=ALU.mult)
    nc.vector.tensor_reduce(out=mxp_col, in_=xw, axis=mybir.AxisListType.X,
                            op=ALU.max, apply_absolute_value=True)


def _nq_finalize(nc, small, sqp, mxp, ws_list, eps_t, D):
    """Combine per-chunk stats (column layout [P,1]) into qm and alphas."""
    ssq = small.tile([P, 1], F32, tag="nq_ssq", name="nq_ssq")
    nc.vector.tensor_reduce(out=ssq, in_=sqp, axis=mybir.AxisListType.X,
                            op=ALU.add)
    tmp = small.tile([P, 1], F32, tag="nq_tmp", name="nq_tmp")
    nc.scalar.activation(out=tmp, in_=ssq, func=AF.Sqrt, scale=1.0 / D,
                         bias=eps_t)
    rstd = small.tile([P, 1], F32, tag="nq_rstd", name="nq_rstd")
    nc.vector.reciprocal(out=rstd, in_=tmp)
    mx = small.tile([P, 1], F32, tag="nq_mx", name="nq_mx")
    nc.vector.tensor_reduce(out=mx, in_=mxp, axis=mybir.AxisListType.X,
                            op=ALU.max)
    s = small.tile([P, 1], F32, tag="nq_s", name="nq_s")
    nc.vector.tensor_scalar(out=s, in0=mx, scalar1=rstd, scalar2=1e-5,
                            op0=ALU.mult, op1=ALU.max)
    rs = small.tile([P, 1], F32, tag="nq_rs", name="nq_rs")
    nc.vector.reciprocal(out=rs, in_=s)
    qm = small.tile([P, 1], F32, tag="nq_qm", name="nq_qm")
    nc.vector.tensor_scalar(out=qm, in0=rs, scalar1=rstd, scalar2=127.0,
                            op0=ALU.mult, op1=ALU.mult)
    alphas = []
    for j, (ws_t, cmul) in enumerate(ws_list):
        a = small.tile([P, 1], F32, tag=f"nq_a{j}", name="nq_aj")
        nc.vector.tensor_scalar(out=a, in0=s, scalar1=ws_t, scalar2=cmul,
                                op0=ALU.mult, op1=ALU.mult)
        alphas.append(a)
    return qm, alphas


def _nq_finalize_row(nc, rows, rwk, ssq_row, mx2_row, ws_row_list, eps_row,
                     D, tag):
    """Row-layout finalize: ssq_row [1,n] sum of squares, mx2_row [1,n]
    max of (x*w)^2.  Returns qm_row [1,n] (f32r) and alpha rows.

    Intermediates come from the shared rwk pool (fixed tags, bufs>=2);
    persistent outputs come from the bufs=1 rows pool (per-call tags)."""
    n = ssq_row.shape[-1]
    tmp = rwk.tile([1, T], F32, tag="ri_tmp", name="r_tmp")[:, :n]
    nc.scalar.activation(out=tmp, in_=ssq_row, func=AF.Sqrt, scale=1.0 / D,
                         bias=eps_row)
    rstd = rwk.tile([1, T], F32, tag="ri_rstd", name="r_rstd")[:, :n]
    nc.vector.reciprocal(out=rstd, in_=tmp)
    mx = rwk.tile([1, T], F32, tag="ri_mx", name="r_mx")[:, :n]
    nc.scalar.activation(out=mx, in_=mx2_row, func=AF.Sqrt)
    s = rwk.tile([1, T], F32, tag="ri_s", name="r_s")[:, :n]
    nc.vector.tensor_tensor(out=s, in0=mx, in1=rstd, op=ALU.mult)
    nc.vector.tensor_scalar(out=s, in0=s, scalar1=1e-5, scalar2=None,
                            op0=ALU.max)
    rs = rwk.tile([1, T], F32, tag="ri_rs", name="r_rs")[:, :n]
    nc.vector.reciprocal(out=rs, in_=s)
    qm_row = rows.tile([1, T], F32R, tag=f"{tag}_qm", name="r_qm")[:, :n]
    nc.vector.tensor_tensor(out=qm_row, in0=rs, in1=rstd, op=ALU.mult)
    nc.vector.tensor_scalar(out=qm_row, in0=qm_row, scalar1=127.0,
                            scalar2=None, op0=ALU.mult)
    arows = []
    for j, (ws_t, cmul) in enumerate(ws_row_list):
        a = rows.tile([1, T], F32, tag=f"{tag}_a{j}", name="r_aj")[:, :n]
        nc.vector.tensor_scalar(out=a, in0=s, scalar1=ws_t[0:1, 0:1],
                                scalar2=cmul, op0=ALU.mult, op1=ALU.mult)
        arows.append(a)
    return qm_row, arows


def _nq_quant_tp(nc, tc, work, src_t, w_b, qms, dstT, ident_bf, pname, D=H):
    """Token-major chunked quantize + PE-transpose into per-block dstT tiles.

    src_t: [TP] list of [P, D] f32.  dstT: list of [P, TP, P] bf16 tiles,
    one per 128-feature block."""
    nch = (D + 511) // 512
    with tc.tile_pool(name=f"psT_{pname}", bufs=2, space="PSUM") as pp:
        for ci in range(nch):
            n0 = ci * 512
            nn = min(512, D - n0)
            for p in range(TP):
                xw = work.tile([P, 512], F32, tag="c512b", name="c512b")[:, :nn]
                nc.gpsimd.tensor_tensor(out=xw, in0=src_t[p][:, n0:n0 + nn],
                                        in1=w_b[:, n0:n0 + nn], op=ALU.mult)
                nc.vector.tensor_scalar(out=xw, in0=xw, scalar1=qms[p],
                                        scalar2=MAGIC,
                                        op0=ALU.mult, op1=ALU.add)
                qc = work.tile([P, 512], BF16, tag="qc", name="qc")[:, :nn]
                nc.vector.tensor_scalar(out=qc, in0=xw, scalar1=-MAGIC,
                                        scalar2=None, op0=ALU.add)
                for bi in range(nn // P):
                    ps = pp.tile([P, P], BF16, tag="t", name="tps")
                    nc.tensor.transpose(ps, qc[:, bi * P:(bi + 1) * P],
                                        ident_bf)
                    dst = dstT[ci * 4 + bi][:, p, :]
                    if bi % 2 == 0:
                        nc.vector.tensor_copy(out=dst, in_=ps)
                    else:
                        nc.scalar.copy(out=dst, in_=ps)


def _proj(nc, wpool, mm, lhsT_fn, w3, kb, n_dim, consume):
    """Unswapped projection: out[t, n] = sum_h lhsT(h,p)^T @ w[h, n].

    lhsT_fn(h, p) -> [128, 128] AP (feature-major activation block).
    w3: [128, kb, n_dim] DRAM view.  consume(p, n0, nn, psum)."""
    for n0 in range(0, n_dim, 512):
        nn = min(512, n_dim - n0)
        ps = [mm.tile([P, 512], F32, tag="acc", name="acc")[:, :nn]
              for p in range(TP)]
        for h0 in range(0, kb, HGRP):
            hg = min(HGRP, kb - h0)
            wt = wpool.tile([P, HGRP, 512], FP8, tag="wt",
                            name="wt")[:, :hg, :nn]
            nc.sync.dma_start(out=wt, in_=w3[:, h0:h0 + hg, n0:n0 + nn])
            for j in range(hg):
                h = h0 + j
                for p in range(TP):
                    nc.tensor.matmul(ps[p], lhsT=lhsT_fn(h, p),
                                     rhs=wt[:, j, :],
                                     start=(h == 0), stop=(h == kb - 1))
        for p in range(TP):
            consume(p, n0, nn, ps[p])


def _emit(nc, tc, aps):
    from contextlib import ExitStack

    isq = 1.0 / np.sqrt(float(HD))

    w3 = {k: aps[k].rearrange("(kb p) n -> p kb n", p=P)
          for k in ("wq", "wk", "wv", "wo", "wg", "wu", "wd")}

    ctx = ExitStack()
    with ctx:
        const = ctx.enter_context(tc.tile_pool(name="const", bufs=1))
        small = ctx.enter_context(tc.tile_pool(name="small", bufs=2))
        work = ctx.enter_context(tc.tile_pool(name="work", bufs=2))
        rows = ctx.enter_context(tc.tile_pool(name="rows", bufs=1))
        rwk = ctx.enter_context(tc.tile_pool(name="rwk", bufs=1))
        wpool = ctx.enter_context(tc.tile_pool(name="wpool", bufs=5))
        dram = ctx.enter_context(tc.tile_pool(name="dram", bufs=1, space="DRAM"))

        # ---------------- constants ----------------
        ws_t = []
        for i in range(5):
            t = const.tile([P, 1], F32, tag=f"wsc{i}", name="wsci")
            _bcast_dma(nc, t, aps["wsc"], i, 1)
            ws_t.append(t)
        wsq_t, wsk_t, wsv_t, wso_t, wsd_t = ws_t

        from concourse.masks import make_identity
        ident = const.tile([P, P], F32, tag="ident", name="ident")
        make_identity(nc, ident)
        ident_bf = const.tile([P, P], BF16, tag="identbf", name="identbf")
        make_identity(nc, ident_bf)
        ones_f = const.tile([P, 2], F32, tag="ones", name="ones")
        nc.vector.memset(ones_f, 1.0)
        ones_r = ones_f[:, 0:1].bitcast(F32R)
        ones_row_f = const.tile([1, P], F32, tag="onesrow", name="onesrow")
        nc.vector.memset(ones_row_f, 1.0)
        ones_row_r = ones_row_f.bitcast(F32R)
        eps_t = const.tile([P, 1], F32, tag="epsc", name="epsc")
        nc.vector.memset(eps_t, EPS)
        eps_row = const.tile([1, 1], F32, tag="epsr", name="epsr")
        nc.vector.memset(eps_row, EPS)

        cosT_sb = const.tile([P, T], F32, tag="cosT", name="cosT")
        sinrT_sb = const.tile([P, T], F32, tag="sinrT", name="sinrT")
        nc.sync.dma_start(out=cosT_sb, in_=aps["cosT"])
        nc.sync.dma_start(out=sinrT_sb, in_=aps["sinrT"])
        wsub_fm = const.tile([P, HB], F32, tag="wsubfm", name="wsubfm")
        nc.sync.dma_start(out=wsub_fm, in_=aps["wsub_fm"])
        wffn_fm = const.tile([P, FB], F32, tag="wffnfm", name="wffnfm")
        nc.sync.dma_start(out=wffn_fm, in_=aps["wffn_fm"])

        sqp_h = [small.tile([P, HB // 4], F32, tag=f"nq_sqph{p}", name="sqph")
                 for p in range(TP)]
        mxp_h = [small.tile([P, HB // 4], F32, tag=f"nq_mxph{p}", name="mxph")
                 for p in range(TP)]
        a_o = [small.tile([P, 1], F32, tag=f"ao{p}", name="aoc")
               for p in range(TP)]

        ag_in = [dram.tile([AGC], F32, name=f"ag_in{g}")
                 for g in range(NKV)]
        ag_out = [dram.tile([NCORES, AGC], F32, name=f"ag_out{g}",
                            addr_space="Shared") for g in range(NKV)]

        hpool = ctx.enter_context(tc.tile_pool(name="hpool", bufs=1))
        wvecp = ctx.enter_context(tc.tile_pool(name="wvecp", bufs=1))
        h_tok = [hpool.tile([P, H], F32, tag=f"h{p}", name="hp")
                 for p in range(TP)]

        with tc.tile_pool(name="opool", bufs=1) as opool:
            # ---------------- input norm + quant ----------------
            nch = HB // 4
            qms, a_q, a_k, a_v = [], [], [], []
            with tc.tile_pool(name="kvq", bufs=1) as kvq:
                qT = kvq.tile([P, NH, TP, P], F32R, tag="qT", name="qT")

                with tc.tile_pool(name="xqp", bufs=1) as xqp, \
                     tc.tile_pool(name="ropec", bufs=1) as ropec, \
                     tc.tile_pool(name="rwork", bufs=2) as rwork:
                    xp_ctx = tc.tile_pool(name="xpool", bufs=1)
                    xpool = xp_ctx.__enter__()
                    x_t = [xpool.tile([P, H], F32, tag=f"x{p}", name="xp")
                           for p in range(TP)]
                    wln_b = xpool.tile([P, H], F32, tag="wvec", name="wvec")
                    _bcast_dma(nc, wln_b, aps["wln"], 0, H)
                    kT_own = xqp.tile([P, NKV, TP, P], F32, tag="kT",
                                      name="kT")
                    v_t = [xqp.tile([P, KV], F32, tag=f"v{p}", name="vp")
                           for p in range(TP)]
                    for p in range(TP):
                        nc.sync.dma_start(out=x_t[p],
                                          in_=aps["x"][p * P:(p + 1) * P, :])
                    for p in range(TP):
                        sqp = small.tile([P, nch], F32, tag="nq_sqp",
                                         name="nq_sqp")
                        mxp = small.tile([P, nch], F32, tag="nq_mxp",
                                         name="nq_mxp")
                        for ci in range(nch):
                            n0 = ci * 512
                            _nq_stats_chunk(nc, work, x_t[p][:, n0:n0 + 512],
                                            wln_b[:, n0:n0 + 512],
                                            sqp[:, ci:ci + 1],
                                            mxp[:, ci:ci + 1],
                                            use_gps=(ci % 2 == 1))
                        qm, al = _nq_finalize(nc, small, sqp, mxp,
                                              [(wsq_t, isq / 127.0),
                                               (wsk_t, 1.0 / 127.0),
                                               (wsv_t, 1.0 / 127.0)],
                                              eps_t, H)
                        qms.append(qm)
                        a_q.append(al[0]); a_k.append(al[1]); a_v.append(al[2])
                    xqT = [xqp.tile([P, TP, P], BF16, tag=f"xq{b}", name="xqb")
                           for b in range(HB)]
                    _nq_quant_tp(nc, tc, work, x_t, wln_b, qms, xqT, ident_bf,
                                 "xq")
                    xp_ctx.__exit__(None, None, None)

                    # rope tables pre-scaled by per-token alphas
                    psB_ctx = tc.tile_pool(name="psB", bufs=2, space="PSUM")
                    psB = psB_ctx.__enter__()
                    arow = {}
                    for nm, acols in (("q", a_q), ("k", a_k)):
                        row = rwk.tile([1, T], F32R, tag=f"arow{nm}",
                                         name="arow")
                        for p in range(TP):
                            pst = psB.tile([1, P], F32, tag="tr", name="tr")
                            nc.tensor.transpose(pst, acols[p], ident)
                            nc.vector.tensor_copy(
                                out=row[:, p * P:(p + 1) * P], in_=pst)
                        arow[nm] = row
                    cos_s, sinr_s = {}, {}
                    for nm in ("q", "k"):
                        ps = psB.tile([P, T], F32, tag="ab", name="ab_ps")
                        nc.tensor.matmul(ps, lhsT=ones_row_r, rhs=arow[nm],
                                         start=True, stop=True)
                        ab = ropec.tile([P, T], F32, tag="ab", name="abt")
                        nc.vector.tensor_copy(out=ab, in_=ps)
                        c = ropec.tile([P, T], F32, tag=f"cs{nm}", name="cst")
                        nc.vector.tensor_tensor(out=c, in0=cosT_sb, in1=ab,
                                                op=ALU.mult)
                        sn = ropec.tile([P, T], F32, tag=f"sn{nm}", name="snt")
                        nc.vector.tensor_tensor(out=sn, in0=sinrT_sb, in1=ab,
                                                op=ALU.mult)
                        cos_s[nm], sinr_s[nm] = c, sn
                    psB_ctx.__exit__(None, None, None)

                    def rope_head(ps, dst, nm):
                        """ps: [P, T] psum feature-major head -> dst [P,TP,P]."""
                        ks = rwork.tile([P, T], F32, tag="ks", name="ks")
                        nc.vector.tensor_copy(out=ks, in_=ps)
                        krot = rwork.tile([P, T], F32, tag="krot", name="krot")
                        nc.scalar.dma_start(out=krot[0:H64, :],
                                            in_=ks[H64:P, :])
                        nc.scalar.dma_start(out=krot[H64:P, :],
                                            in_=ks[0:H64, :])
                        nc.vector.tensor_tensor(out=ks, in0=ks,
                                                in1=cos_s[nm], op=ALU.mult)
                        nc.vector.tensor_tensor(out=krot, in0=krot,
                                                in1=sinr_s[nm], op=ALU.mult)
                        nc.vector.tensor_tensor(
                            out=dst.rearrange("d p t -> d (p t)"),
                            in0=ks, in1=krot, op=ALU.add)

                    # ---- K projection (swapped) + per-head AG K parts ----
                    with tc.tile_pool(name="wkp", bufs=2) as wkp, \
                         tc.tile_pool(name="mmKQ", bufs=4, space="PSUM") as mmKQ:
                        for g in range(NKV):
                            wkt = wkp.tile([P, HB, P], FP8, tag="wkt",
                                           name="wkt")
                            nc.sync.dma_start(
                                out=wkt,
                                in_=w3["wk"][:, :, g * P:(g + 1) * P])
                            ps = mmKQ.tile([P, T], F32, tag="kq", name="kq_ps")
                            for h in range(HB):
                                nc.tensor.matmul(
                                    ps, lhsT=wkt[:, h, :],
                                    rhs=xqT[h].rearrange("d p t -> d (p t)"),
                                    start=(h == 0), stop=(h == HB - 1))
                            rope_head(ps, kT_own[:, g], "k")
                            nc.gpsimd.dma_start(
                                out=ag_in[g][0:HD * T]
                                    .rearrange("(d t) -> d t", d=P),
                                in_=kT_own[:, g].rearrange("d p t -> d (p t)"))

                        # ---- V projection (unswapped, token-major) ----
                        with tc.tile_pool(name="mmV", bufs=4,
                                          space="PSUM") as mmV:
                            def eat_v(p, n0, nn, ps):
                                nc.vector.tensor_scalar(
                                    out=v_t[p][:, n0:n0 + nn], in0=ps,
                                    scalar1=a_v[p], scalar2=None, op0=ALU.mult)
                            _proj(nc, wpool, mmV,
                                  lambda h, p: xqT[h][:, p, :], w3["wv"],
                                  HB, KV, eat_v)
                        for g in range(NKV):
                            for p in range(TP):
                                nc.gpsimd.dma_start(
                                    out=ag_in[g][HD * T + p * (P * HD):
                                                 HD * T + (p + 1) * (P * HD)]
                                        .rearrange("(t d) -> t d", t=P),
                                    in_=v_t[p][:, g * P:(g + 1) * P])
                            nc.gpsimd.collective_compute(
                                "AllGather", ALU.bypass,
                                replica_groups=[list(range(NCORES))],
                                ins=[ag_in[g].opt()], outs=[ag_out[g].opt()])

                        # ---- Q projection (swapped; overlaps the AGs) ----
                        with tc.tile_pool(name="wqp", bufs=2) as wqp:
                            for c2 in range(10):
                                wqt = wqp.tile([P, HB, 256], FP8, tag="wqt",
                                               name="wqt")
                                nc.sync.dma_start(
                                    out=wqt,
                                    in_=w3["wq"][:, :, c2 * 256:(c2 + 1) * 256])
                                for i2 in range(2):
                                    hd = c2 * 2 + i2
                                    ps = mmKQ.tile([P, T], F32, tag="kq",
                                                   name="kq_ps")
                                    for h in range(HB):
                                        nc.tensor.matmul(
                                            ps,
                                            lhsT=wqt[:, h, i2 * P:(i2 + 1) * P],
                                            rhs=xqT[h].rearrange(
                                                "d p t -> d (p t)"),
                                            start=(h == 0), stop=(h == HB - 1))
                                    rope_head(ps, qT[:, hd], "q")

                # ---------------- attention ----------------
                o_un, oq = {}, {}
                for g in range(NKV):
                    for p in range(TP):
                        o_un[(g, p)] = opool.tile([P, GQ * P], F32,
                                                  tag=f"ou{g}{p}", name="ou")
                        oq[(g, p)] = opool.tile([P, GQ * P], BF16,
                                                tag=f"oq{g}{p}", name="oqt")
                macc_p = [opool.tile([1, GQ * P], F32, tag=f"macc{p}",
                                     name="macc") for p in range(TP)]
                ssq_sb = [opool.tile([1, GQ * P], F32, tag=f"ssqs{p}",
                                     name="ssqs") for p in range(TP)]
                qm_b = [opool.tile([P, P], F32, tag=f"qmb{p}", name="qmbt")
                        for p in range(TP)]

                with tc.tile_pool(name="attsb", bufs=2) as attp, \
                     tc.tile_pool(name="maskp", bufs=1) as maskp, \
                     tc.tile_pool(name="ptp", bufs=4) as ptp, \
                     tc.tile_pool(name="att2", bufs=2) as att2, \
                     tc.tile_pool(name="awork", bufs=1) as awork, \
                     tc.tile_pool(name="psS", bufs=2, space="PSUM") as psS, \
                     tc.tile_pool(name="psA", bufs=2, space="PSUM") as psA, \
                     tc.tile_pool(name="psD", bufs=2, space="PSUM") as psD, \
                     tc.tile_pool(name="psQ", bufs=2, space="PSUM") as psQ:
                    mask_sb = maskp.tile([P, NCORES, P], BF16, tag="mask",
                                        name="mask")
                    nc.sync.dma_start(out=mask_sb,
                                      in_=aps["mask"].rearrange("r k q -> k r q"))
                    sq_ps = [psQ.tile([1, GQ * P], F32, tag="sq", name="sq_ps")
                             for p in range(TP)]
                    for g in range(NKV):
                        K_g = attp.tile([P, NCORES, TP, P], F32, tag="K",
                                        name="Kg")
                        V_g = attp.tile([P, NCORES, TP, P], F32, tag="V",
                                        name="Vg")
                        for r in range(NCORES):
                            nc.scalar.dma_start(
                                out=K_g[:, r].rearrange("d p t -> d (p t)"),
                                in_=ag_out[g][r, 0:HD * T]
                                    .rearrange("(d t) -> d t", d=P))
                            nc.scalar.dma_start(
                                out=V_g[:, r],
                                in_=ag_out[g][r, HD * T:]
                                    .rearrange("(p t d) -> t p d",
                                               p=TP, t=P))
                        for p in range(TP):
                            ps_att = psA.tile([P, GQ * P], F32, tag="att",
                                              name="att")
                            ps_den = psD.tile([1, GQ * P], F32, tag="den",
                                              name="den")
                            nk = NCORES * (p + 1)
                            idx = 0
                            for h in range(p + 1):
                                for r in range(NCORES):
                                    ps_s = psS.tile([P, GQ * P], F32, tag="s",
                                                    name="s")
                                    nc.tensor.matmul(
                                        ps_s,
                                        lhsT=K_g[:, r, h, :].bitcast(F32R),
                                        rhs=qT[:, GQ * g:GQ * (g + 1), p, :],
                                        start=True, stop=True)
                                    if h == p:
                                        v3 = ps_s.rearrange(
                                            "a (i q) -> a i q", i=GQ)
                                        nc.vector.tensor_tensor(
                                            out=v3, in0=v3,
                                            in1=mask_sb[:, r, None, :]
                                                .to_broadcast((P, GQ, P)),
                                            op=ALU.add)
                                    pt = ptp.tile([P, GQ * P], F32R, tag="pt",
                                                  name="pt")
                                    nc.scalar.activation(out=pt, in_=ps_s,
                                                         func=AF.Exp)
                                    nc.tensor.matmul(
                                        ps_att,
                                        lhsT=V_g[:, r, h, :].bitcast(F32R),
                                        rhs=pt,
                                        start=(idx == 0),
                                        stop=(idx == nk - 1))
                                    nc.tensor.matmul(
                                        ps_den, lhsT=ones_r,
                                        rhs=pt,
                                        start=(idx == 0), stop=(idx == nk - 1))
                                    idx += 1
                            # normalize feature-major: o_n = ps_att / den
                            den_r = att2.tile([1, GQ * P], F32, tag="denr",
                                              name="denr")
                            nc.vector.tensor_copy(out=den_r, in_=ps_den)
                            rden = att2.tile([1, GQ * P], F32R, tag="rden",
                                             name="rden")
                            with nc.allow_low_precision(
                                    reason="f32r is bit-identical to f32"):
                                nc.vector.reciprocal(out=rden, in_=den_r)
                            ps_b = psS.tile([P, GQ * P], F32, tag="s",
                                            name="rdb")
                            nc.tensor.matmul(ps_b, lhsT=ones_row_r, rhs=rden,
                                             start=True, stop=True)
                            rden_b = awork.tile([P, GQ * P], F32, tag="rdenb",
                                                name="rdenb")
                            nc.vector.tensor_copy(out=rden_b, in_=ps_b)
                            ou = o_un[(g, p)]
                            nc.vector.tensor_tensor(out=ou, in0=ps_att,
                                                    in1=rden_b, op=ALU.mult)
                            # stats: sum-of-squares via ones-matmul (PSUM
                            # accumulated across g); absmax via gpsimd
                            sq = awork.tile([P, GQ * P], F32R, tag="sqw",
                                           name="sqw")
                            nc.scalar.activation(out=sq, in_=ou,
                                                 func=AF.Square)
                            nc.tensor.matmul(sq_ps[p], lhsT=ones_r,
                                             rhs=sq,
                                             start=(g == 0),
                                             stop=(g == NKV - 1))
                            mw = awork.tile([P, GQ * P], F32, tag="mww",
                                           name="mww")
                            nc.vector.tensor_tensor(
                                out=mw.rearrange("d (i q) -> d i q", i=GQ),
                                in0=ou.rearrange("d (i q) -> d i q", i=GQ),
                                in1=wsub_fm[:, GQ * g:GQ * (g + 1), None]
                                    .to_broadcast((P, GQ, P)),
                                op=ALU.mult)
                            mw2 = awork.tile([P, GQ * P], F32, tag="mw2",
                                            name="mw2")
                            nc.scalar.activation(out=mw2, in_=mw,
                                                 func=AF.Square)
                            red = awork.tile([P, GQ * P], F32, tag="red",
                                            name="red")
                            nc.gpsimd.partition_all_reduce(
                                red, mw2, channels=P,
                                reduce_op=bass_isa.ReduceOp.max)
                            if g == 0:
                                nc.vector.tensor_copy(out=macc_p[p],
                                                      in_=red[0:1, :])
                            else:
                                nc.vector.tensor_tensor(out=macc_p[p],
                                                        in0=macc_p[p],
                                                        in1=red[0:1, :],
                                                        op=ALU.max)

                    # spill the PSUM sum-of-squares rows before pools close
                    for p in range(TP):
                        nc.vector.tensor_copy(out=ssq_sb[p], in_=sq_ps[p])

            # ---- residual reload + o finalize + feature-major quant ----
            x2p_ctx = tc.tile_pool(name="x2pool", bufs=1)
            x2pool = x2p_ctx.__enter__()
            x2_t = [x2pool.tile([P, H], F32, tag=f"x2{p}", name="x2p")
                    for p in range(TP)]
            for p in range(TP):
                nc.sync.dma_start(out=x2_t[p],
                                  in_=aps["x"][p * P:(p + 1) * P, :])
            with tc.tile_pool(name="fwork", bufs=2) as fwork, \
                 tc.tile_pool(name="psF", bufs=2, space="PSUM") as psF:
                    # ---------- o finalize + feature-major quant ----------
                    for p in range(TP):
                        ssq_iq = ssq_sb[p]
                        ssq_row = fwork.tile([1, P], F32, tag="ssqr",
                                             name="ssqr")
                        nc.vector.tensor_reduce(
                            out=ssq_row,
                            in_=ssq_iq.rearrange("a (i q) -> a q i", i=GQ),
                            axis=mybir.AxisListType.X, op=ALU.add)
                        mx2_row = fwork.tile([1, P], F32, tag="mx2r",
                                             name="mx2r")
                        nc.vector.tensor_reduce(
                            out=mx2_row,
                            in_=macc_p[p].rearrange("a (i q) -> a q i", i=GQ),
                            axis=mybir.AxisListType.X, op=ALU.max)
                        qm_row, arows = _nq_finalize_row(
                            nc, rows, rwk, ssq_row, mx2_row,
                            [(wso_t, 1.0 / 127.0)], eps_row, H, f"of{p}")
                        ps_b = psF.tile([P, GQ * P], F32, tag="s",
                                        name="qmb")[:, :P]
                        nc.tensor.matmul(ps_b, lhsT=ones_row_r, rhs=qm_row,
                                         start=True, stop=True)
                        nc.vector.tensor_copy(out=qm_b[p], in_=ps_b)
                        # a_o back to column form for the o-proj eat
                        a_row_r = fwork.tile([1, P], F32R, tag="aor",
                                             name="aor")
                        nc.vector.tensor_copy(out=a_row_r, in_=arows[0])
                        ps_t = psF.tile([P, GQ * P], F32, tag="s",
                                        name="aot")[:, :2]
                        nc.tensor.matmul(ps_t, lhsT=a_row_r,
                                         rhs=ones_row_r[0:1, 0:2],
                                         start=True, stop=True)
                        nc.vector.tensor_copy(out=a_o[p], in_=ps_t[:, 0:1])
                    for g in range(NKV):
                        for p in range(TP):
                            u1 = fwork.tile([P, GQ * P], F32, tag="mww",
                                           name="u1")
                            nc.vector.tensor_tensor(
                                out=u1.rearrange("d (i q) -> d i q", i=GQ),
                                in0=o_un[(g, p)].rearrange(
                                    "d (i q) -> d i q", i=GQ),
                                in1=wsub_fm[:, GQ * g:GQ * (g + 1), None]
                                    .to_broadcast((P, GQ, P)),
                                op=ALU.mult)
                            u2 = fwork.tile([P, GQ * P], F32, tag="mw2",
                                           name="u2")
                            nc.vector.tensor_tensor(
                                out=u2.rearrange("d (i q) -> d i q", i=GQ),
                                in0=u1.rearrange("d (i q) -> d i q", i=GQ),
                                in1=qm_b[p][:, None, :]
                                    .to_broadcast((P, GQ, P)),
                                op=ALU.mult)
                            nc.vector.tensor_scalar(out=oq[(g, p)], in0=u2,
                                                    scalar1=MAGIC,
                                                    scalar2=-MAGIC,
                                                    op0=ALU.add, op1=ALU.add)

            # ---------------- o-proj ----------------
            wpost_b = wvecp.tile([P, H], F32, tag="wvec", name="wvec")
            _bcast_dma(nc, wpost_b, aps["wpost"], 0, H)
            with tc.tile_pool(name="mmO", bufs=4, space="PSUM") as mmO:
                def eat_o(p, n0, nn, ps):
                    sl = h_tok[p][:, n0:n0 + nn]
                    nc.vector.tensor_scalar(out=sl, in0=ps, scalar1=a_o[p],
                                            scalar2=None, op0=ALU.mult)
                    nc.vector.tensor_add(out=sl, in0=sl,
                                         in1=x2_t[p][:, n0:n0 + nn])
                    ci = n0 // 512
                    _nq_stats_chunk(nc, work, sl, wpost_b[:, n0:n0 + nn],
                                    sqp_h[p][:, ci:ci + 1],
                                    mxp_h[p][:, ci:ci + 1],
                                    use_gps=(ci % 2 == 1))
                _proj(nc, wpool, mmO,
                      lambda h, p: oq[(h // GQ, p)][:, (h % GQ) * P:
                                                    (h % GQ + 1) * P],
                      w3["wo"], HB, H, eat_o)
            x2p_ctx.__exit__(None, None, None)

        # ---------------- MLP ----------------
        qms_2 = []
        for p in range(TP):
            qm, _ = _nq_finalize(nc, small, sqp_h[p], mxp_h[p], [], eps_t, H)
            qms_2.append(qm)

        with tc.tile_pool(name="xq2p", bufs=1) as xq2p, \
             tc.tile_pool(name="mpool", bufs=1) as mpool:
            xq2T = [xq2p.tile([P, TP, P], BF16, tag=f"x2{b}", name="x2b")
                    for b in range(HB)]
            _nq_quant_tp(nc, tc, work, h_tok, wpost_b, qms_2, xq2T, ident_bf,
                         "xq2")

            m_fm = [mpool.tile([P, T], F32, tag=f"m{b}", name="mb")
                    for b in range(FB)]
            macc2 = mpool.tile([P, T], F32, tag="macc2", name="macc2")

            with tc.tile_pool(name="wgup", bufs=2) as wgup, \
                 tc.tile_pool(name="mwork", bufs=1) as mwork, \
                 tc.tile_pool(name="psG", bufs=4, space="PSUM") as psG, \
                 tc.tile_pool(name="psQ2", bufs=1, space="PSUM") as psQ2:
                sq2_ps = psQ2.tile([1, T], F32, tag="sq2", name="sq2")
                for fb4 in range(0, FB, 2):
                    nfb = min(2, FB - fb4)
                    wtg = wgup.tile([P, HB, 256], FP8, tag="wtg",
                                    name="wtg")[:, :, :nfb * P]
                    wtu = wgup.tile([P, HB, 256], FP8, tag="wtu",
                                    name="wtu")[:, :, :nfb * P]
                    nc.sync.dma_start(
                        out=wtg, in_=w3["wg"][:, :, fb4 * P:(fb4 + nfb) * P])
                    nc.sync.dma_start(
                        out=wtu, in_=w3["wu"][:, :, fb4 * P:(fb4 + nfb) * P])
                    for j in range(nfb):
                        fb = fb4 + j
                        ps_g = psG.tile([P, T], F32, tag="gu", name="gps")
                        ps_u = psG.tile([P, T], F32, tag="gu", name="ups")
                        for h in range(HB):
                            rhs = xq2T[h].rearrange("d p t -> d (p t)")
                            nc.tensor.matmul(
                                ps_g, lhsT=wtg[:, h, j * P:(j + 1) * P],
                                rhs=rhs, start=(h == 0), stop=(h == HB - 1))
                            nc.tensor.matmul(
                                ps_u, lhsT=wtu[:, h, j * P:(j + 1) * P],
                                rhs=rhs, start=(h == 0), stop=(h == HB - 1))
                        gr = mwork.tile([P, T], F32, tag="gr", name="gr")
                        nc.vector.tensor_scalar(out=gr, in0=ps_g, scalar1=0.0,
                                                scalar2=None, op0=ALU.max)
                        gr2 = mwork.tile([P, T], F32, tag="gr2", name="gr2")
                        nc.scalar.activation(out=gr2, in_=gr, func=AF.Square)
                        mf = m_fm[fb]
                        nc.vector.tensor_tensor(out=mf, in0=gr2, in1=ps_u,
                                                op=ALU.mult)
                        mw = mwork.tile([P, T], F32, tag="mwf", name="mwf")
                        nc.vector.tensor_scalar(
                            out=mw, in0=mf, scalar1=wffn_fm[:, fb:fb + 1],
                            scalar2=None, op0=ALU.mult)
                        mw2 = mwork.tile([P, T], F32, tag="mw2f", name="mw2f")
                        nc.scalar.activation(out=mw2, in_=mw, func=AF.Square)
                        if fb == 0:
                            nc.vector.tensor_copy(out=macc2, in_=mw2)
                        else:
                            nc.vector.tensor_tensor(out=macc2, in0=macc2,
                                                    in1=mw2, op=ALU.max)
                        m2 = mwork.tile([P, T], F32R, tag="m2f", name="m2f")
                        nc.scalar.activation(out=m2, in_=mf, func=AF.Square)
                        nc.tensor.matmul(sq2_ps, lhsT=ones_r,
                                         rhs=m2,
                                         start=(fb == 0), stop=(fb == FB - 1))

                # m finalize (row form)
                red2 = mpool.tile([P, T], F32, tag="red2", name="red2")
                nc.gpsimd.partition_all_reduce(
                    red2, macc2, channels=P, reduce_op=bass_isa.ReduceOp.max)
                ssq2_row = rwk.tile([1, T], F32, tag="ssq2", name="ssq2")
                nc.vector.tensor_copy(out=ssq2_row, in_=sq2_ps)
                qm2_row, arows2 = _nq_finalize_row(
                    nc, rows, rwk, ssq2_row, red2[0:1, :],
                    [(wsd_t, 1.0 / 127.0)], eps_row, FF, "mf")
                with tc.tile_pool(name="psM", bufs=2, space="PSUM") as psM:
                    ps_b = psM.tile([P, T], F32, tag="b", name="qm2b")
                    nc.tensor.matmul(ps_b, lhsT=ones_row_r, rhs=qm2_row,
                                     start=True, stop=True)
                    qm2_b = mpool.tile([P, T], F32, tag="qm2b", name="qm2bt")
                    nc.vector.tensor_copy(out=qm2_b, in_=ps_b)
                    a_d = []
                    a2r = rwk.tile([1, T], F32R, tag="a2r", name="a2r")
                    nc.vector.tensor_copy(out=a2r, in_=arows2[0])
                    for p in range(TP):
                        ps_t = psM.tile([P, T], F32, tag="b",
                                        name="adt")[:, :2]
                        nc.tensor.matmul(ps_t,
                                         lhsT=a2r[:, p * P:(p + 1) * P],
                                         rhs=ones_row_r[0:1, 0:2],
                                         start=True, stop=True)
                        ac = small.tile([P, 1], F32, tag=f"ad{p}", name="adc")
                        nc.vector.tensor_copy(out=ac, in_=ps_t[:, 0:1])
                        a_d.append(ac)

            # m quantize (feature-major, no transposes)
            mq = [mpool.tile([P, T], BF16, tag=f"mq{b}", name="mqb")
                  for b in range(FB)]
            with tc.tile_pool(name="qwork", bufs=1) as qwork:
                for fb in range(FB):
                    u1 = qwork.tile([P, T], F32, tag="mwf", name="u1m")
                    nc.vector.tensor_scalar(out=u1, in0=m_fm[fb],
                                            scalar1=wffn_fm[:, fb:fb + 1],
                                            scalar2=None, op0=ALU.mult)
                    u2 = qwork.tile([P, T], F32, tag="mw2f", name="u2m")
                    nc.vector.tensor_tensor(out=u2, in0=u1, in1=qm2_b,
                                            op=ALU.mult)
                    nc.vector.tensor_scalar(out=mq[fb], in0=u2, scalar1=MAGIC,
                                            scalar2=-MAGIC, op0=ALU.add,
                                            op1=ALU.add)

            # down-proj
            with tc.tile_pool(name="mmF", bufs=4, space="PSUM") as mmF:
                def eat_d(p, n0, nn, ps):
                    o_sb = work.tile([P, 512], F32, tag="c512a",
                                     name="osb")[:, :nn]
                    nc.vector.tensor_scalar(out=o_sb, in0=ps, scalar1=a_d[p],
                                            scalar2=None, op0=ALU.mult)
                    nc.vector.tensor_add(out=o_sb, in0=o_sb,
                                         in1=h_tok[p][:, n0:n0 + nn])
                    nc.sync.dma_start(out=aps["out"][p * P:(p + 1) * P,
                                                     n0:n0 + nn],
                                      in_=o_sb)
                _proj(nc, wpool, mmF,
                      lambda h, p: mq[h][:, p * P:(p + 1) * P],
                      w3["wd"], FB, H, eat_d)


_NC_CACHE = {}


def _get_nc():
    if "nc" not in _NC_CACHE:
        _NC_CACHE["nc"] = _build_nc()
    return _NC_CACHE["nc"]


def _quant_w(w):
    w = np.asarray(w, np.float32)
    ws = np.maximum(np.float32(np.abs(w).mean(dtype=np.float32)),
                    np.float32(1e-5))
    wq = np.clip(np.round(w / ws), -1.0, 1.0).astype(np.float32)
    return wq, float(ws)


def kernel(hidden_states, cos, sin, w_in_ln, w_q, w_k, w_v, w_o,
           w_attn_sub, w_post_ln, w_gate, w_up, w_ffn_sub, w_down,
           _trace=False):
    hs = np.asarray(hidden_states, np.float32)
    assert hs.shape == (1, S, H)

    nc = _get_nc()

    wq_i, s_q = _quant_w(w_q)
    wk_i, s_k = _quant_w(w_k)
    wv_i, s_v = _quant_w(w_v)
    wo_i, s_o = _quant_w(w_o)
    wg_i, _ = _quant_w(w_gate)
    wu_i, _ = _quant_w(w_up)
    wd_i, s_d = _quant_w(w_down)

    f8 = ml_dtypes.float8_e4m3
    shared = {
        "wq": np.ascontiguousarray(wq_i.T).astype(f8),
        "wk": np.ascontiguousarray(wk_i.T).astype(f8),
        "wv": np.ascontiguousarray(wv_i.T).astype(f8),
        "wo": np.ascontiguousarray(wo_i.T).astype(f8),
        "wg": np.ascontiguousarray(wg_i.T).astype(f8),
        "wu": np.ascontiguousarray(wu_i.T).astype(f8),
        "wd": np.ascontiguousarray(wd_i.T).astype(f8),
        "wln": np.asarray(w_in_ln, np.float32),
        "wsub_fm": np.ascontiguousarray(
            np.asarray(w_attn_sub, np.float32).reshape(HB, P).T),
        "wpost": np.asarray(w_post_ln, np.float32),
        "wffn_fm": np.ascontiguousarray(
            np.asarray(w_ffn_sub, np.float32).reshape(FB, P).T),
        "wsc": np.array([s_q, s_k, s_v, s_o, s_d], np.float32),
    }

    cos0 = np.asarray(cos, np.float32)[0]    # [S, HD]
    sin0 = np.asarray(sin, np.float32)[0]
    sinr = sin0.copy()
    sinr[:, :HD // 2] = -sin0[:, :HD // 2]

    x_resh = hs[0].reshape(T, NCORES, H)
    cos_resh = cos0.reshape(T, NCORES, HD)
    sinr_resh = sinr.reshape(T, NCORES, HD)

    kk, qq = np.meshgrid(np.arange(P), np.arange(P), indexing="ij")
    in_maps = []
    for c in range(NCORES):
        masks = np.empty((NCORES, P, P), np.float32)
        for r in range(NCORES):
            lim = qq - (1 if r > c else 0)
            masks[r] = np.where(kk <= lim, 0.0, NEG)
        m = dict(shared)
        m["x"] = np.ascontiguousarray(x_resh[:, c, :])
        m["cosT"] = np.ascontiguousarray(cos_resh[:, c, :].T)
        m["sinrT"] = np.ascontiguousarray(sinr_resh[:, c, :].T)
        m["mask"] = masks.astype(ml_dtypes.bfloat16)
        in_maps.append(m)

    res = bass_utils.run_bass_kernel_spmd(
        nc, in_maps, core_ids=list(range(NCORES)), trace=_trace)

    out = np.empty((1, S, H), np.float32)
    out_resh = out[0].reshape(T, NCORES, H)
    for c in range(NCORES):
        out_resh[:, c, :] = res.results[c]["out"]

    kernel._last_results = res
    return out


# revision 34
# speedup vs baseline: 1.0696x; 1.0696x over previous
"""BitNet transformer layer (B=1, S=2048, H=2560, NH=20, NKV=5, HD=128, FF=6912)
on 8 Trainium2 NeuronCores.

Sharding: sequence-interleaved data parallel. Core c owns tokens {8*i + c}.
All weights are replicated (ternary-quantized on host to exact {-1,0,+1} fp8,
so every projection matmul is integer-exact with fp32 PSUM accumulation).

v2 design notes:
- fp32(r) attention path end to end (q/k/v/probs) - fp32r matmuls run at
  bf16 rate for moving dims >= 256, and the extra precision keeps the
  int8-quantizer rounding cliffs downstream from amplifying bf16 noise.
- Q/K projections run operand-swapped (weights stationary) so q^T/k^T come
  out of the PE feature-major directly; RoPE is applied feature-major with
  the half-rotation materialized by two PSUM->SBUF DMAs.
- Per-kv-head AllGather (5 collectives) so attention can start as soon as
  the first head's K/V has arrived; each AG overlaps the Q projection.
- o and m (down-proj input) are normalized+quantized feature-major with
  per-column scales (broadcast via tiny PE outer products), eliminating
  all o/m quant transposes and the softmax-denominator transposes.
- gate/up run operand-swapped producing feature-major m directly.
"""

import sys

import numpy as np

if "/opt/trn_rl_repo" not in sys.path:
    sys.path.insert(0, "/opt/trn_rl_repo")

import ml_dtypes

import concourse.bass as bass
import concourse.tile as tile
from concourse import bacc, mybir
from concourse import bass_utils
from concourse import bass_isa

F32 = mybir.dt.float32
F32R = mybir.dt.float32r
F16 = mybir.dt.float16
BF16 = mybir.dt.bfloat16
FP8 = mybir.dt.float8e4
AF = mybir.ActivationFunctionType
ALU = mybir.AluOpType

NCORES = 8
S, H, NH, NKV, HD, FF = 2048, 2560, 20, 5, 128, 6912
T = S // NCORES            # 256 tokens per core
P = 128
TP = T // P                # 2 token ptiles per core
HB = H // P                # 20 hidden blocks
FB = FF // P               # 54 ff blocks
GQ = NH // NKV             # 4 q heads per kv head
KV = NKV * HD              # 640
EPS = 1e-5
MAGIC = 12582912.0         # 1.5*2^23: (x+MAGIC)-MAGIC == rne-round(x) for |x|<2^22
NEG = -1e30
HGRP = 10                  # contraction blocks per weight macro-DMA
H64 = HD // 2
AGC = HD * T + T * HD      # elems per core per AG section (K part + V part)


def _bcast_dma(nc, out_tile, dram_ap, offset_elems, n):
    """DMA a [n] f32 DRAM vector to [128, n] SBUF, broadcast over partitions."""
    src = bass.AP(tensor=dram_ap.tensor, offset=offset_elems, ap=[[0, P], [1, n]])
    nc.gpsimd.dma_start(out=out_tile, in_=src)


def _build_nc():
    nc = bacc.Bacc("TRN2", target_bir_lowering=False, debug=False,
                   num_devices=NCORES)

    aps = {}
    def inp(name, shape, dt):
        aps[name] = nc.dram_tensor(name, shape, dt, kind="ExternalInput").ap()
    inp("x", [T, H], F32)
    inp("cosT", [HD, T], F32)     # feature-major rope tables
    inp("sinrT", [HD, T], F32)    # sign-folded: rows 0..63 negated
    inp("mask", [NCORES, P, P], BF16)
    inp("wq", [H, H], FP8)
    inp("wk", [H, KV], FP8)
    inp("wv", [H, KV], FP8)
    inp("wo", [H, H], FP8)
    inp("wg", [H, FF], FP8)
    inp("wu", [H, FF], FP8)
    inp("wd", [FF, H], FP8)
    inp("wln", [H], F32)
    inp("wsub_fm", [P, HB], F32)   # w_attn_sub feature-major [d, block]
    inp("wpost", [H], F32)
    inp("wffn_fm", [P, FB], F32)   # w_ffn_sub feature-major [d, block]
    inp("wsc", [5], F32)   # [wsq, wsk, wsv, wso, wsd]
    aps["out"] = nc.dram_tensor("out", [T, H], F32, kind="ExternalOutput").ap()

    with tile.TileContext(nc) as tc:
        _emit(nc, tc, aps)

    nc.compile()
    return nc


def _nq_stats_chunk(nc, work, src_slice, w_b_slice, sqp_col, mxp_col, use_gps):
    """Square-accum + |x*w| max for one [128, nn] chunk."""
    nn = src_slice.shape[-1]
    scr = work.tile([P, 512], F32, tag="c512a", name="c512a")[:, :nn]
    nc.scalar.activation(out=scr, in_=src_slice, func=AF.Square,
                         accum_out=sqp_col)
    xw = work.tile([P, 512], F32, tag="c512b", name="c512b")[:, :nn]
    eng = nc.gpsimd if use_gps else nc.vector
    eng.tensor_tensor(out=xw, in0=src_slice, in1=w_b_slice, op=ALU.mult)
    nc.vector.tensor_reduce(out=mxp_col, in_=xw, axis=mybir.AxisListType.X,
                            op=ALU.max, apply_absolute_value=True)


def _nq_finalize(nc, small, sqp, mxp, ws_list, eps_t, D):
    """Combine per-chunk stats (column layout [P,1]) into qm and alphas."""
    ssq = small.tile([P, 1], F32, tag="nq_ssq", name="nq_ssq")
    nc.vector.tensor_reduce(out=ssq, in_=sqp, axis=mybir.AxisListType.X,
                            op=ALU.add)
    tmp = small.tile([P, 1], F32, tag="nq_tmp", name="nq_tmp")
    nc.scalar.activation(out=tmp, in_=ssq, func=AF.Sqrt, scale=1.0 / D,
                         bias=eps_t)
    rstd = small.tile([P, 1], F32, tag="nq_rstd", name="nq_rstd")
    nc.vector.reciprocal(out=rstd, in_=tmp)
    mx = small.tile([P, 1], F32, tag="nq_mx", name="nq_mx")
    nc.vector.tensor_reduce(out=mx, in_=mxp, axis=mybir.AxisListType.X,
                            op=ALU.max)
    s = small.tile([P, 1], F32, tag="nq_s", name="nq_s")
    nc.vector.tensor_scalar(out=s, in0=mx, scalar1=rstd, scalar2=1e-5,
                            op0=ALU.mult, op1=ALU.max)
    rs = small.tile([P, 1], F32, tag="nq_rs", name="nq_rs")
    nc.vector.reciprocal(out=rs, in_=s)
    qm = small.tile([P, 1], F32, tag="nq_qm", name="nq_qm")
    nc.vector.tensor_scalar(out=qm, in0=rs, scalar1=rstd, scalar2=127.0,
                            op0=ALU.mult, op1=ALU.mult)
    alphas = []
    for j, (ws_t, cmul) in enumerate(ws_list):
        a = small.tile([P, 1], F32, tag=f"nq_a{j}", name="nq_aj")
        nc.vector.tensor_scalar(out=a, in0=s, scalar1=ws_t, scalar2=cmul,
                                op0=ALU.mult, op1=ALU.mult)
        alphas.append(a)
    return qm, alphas


def _nq_finalize_row(nc, rows, rwk, ssq_row, mx2_row, ws_row_list, eps_row,
                     D, tag):
    """Row-layout finalize: ssq_row [1,n] sum of squares, mx2_row [1,n]
    max of (x*w)^2.  Returns qm_row [1,n] (f32r) and alpha rows.

    Intermediates come from the shared rwk pool (fixed tags, bufs>=2);
    persistent outputs come from the bufs=1 rows pool (per-call tags)."""
    n = ssq_row.shape[-1]
    tmp = rwk.tile([1, T], F32, tag="ri_tmp", name="r_tmp")[:, :n]
    nc.scalar.activation(out=tmp, in_=ssq_row, func=AF.Sqrt, scale=1.0 / D,
                         bias=eps_row)
    rstd = rwk.tile([1, T], F32, tag="ri_rstd", name="r_rstd")[:, :n]
    nc.vector.reciprocal(out=rstd, in_=tmp)
    mx = rwk.tile([1, T], F32, tag="ri_mx", name="r_mx")[:, :n]
    nc.scalar.activation(out=mx, in_=mx2_row, func=AF.Sqrt)
    s = rwk.tile([1, T], F32, tag="ri_s", name="r_s")[:, :n]
    nc.vector.tensor_tensor(out=s, in0=mx, in1=rstd, op=ALU.mult)
    nc.vector.tensor_scalar(out=s, in0=s, scalar1=1e-5, scalar2=None,
                            op0=ALU.max)
    rs = rwk.tile([1, T], F32, tag="ri_rs", name="r_rs")[:, :n]
    nc.vector.reciprocal(out=rs, in_=s)
    qm_row = rows.tile([1, T], F32R, tag=f"{tag}_qm", name="r_qm")[:, :n]
    nc.vector.tensor_tensor(out=qm_row, in0=rs, in1=rstd, op=ALU.mult)
    nc.vector.tensor_scalar(out=qm_row, in0=qm_row, scalar1=127.0,
                            scalar2=None, op0=ALU.mult)
    arows = []
    for j, (ws_t, cmul) in enumerate(ws_row_list):
        a = rows.tile([1, T], F32, tag=f"{tag}_a{j}", name="r_aj")[:, :n]
        nc.vector.tensor_scalar(out=a, in0=s, scalar1=ws_t[0:1, 0:1],
                                scalar2=cmul, op0=ALU.mult, op1=ALU.mult)
        arows.append(a)
    return qm_row, arows


def _nq_quant_tp(nc, tc, work, src_t, w_b, qms, dstT, ident_bf, pname, D=H):
    """Token-major chunked quantize + PE-transpose into per-block dstT tiles.

    src_t: [TP] list of [P, D] f32.  dstT: list of [P, TP, P] bf16 tiles,
    one per 128-feature block."""
    nch = (D + 511) // 512
    with tc.tile_pool(name=f"psT_{pname}", bufs=2, space="PSUM") as pp:
        for ci in range(nch):
            n0 = ci * 512
            nn = min(512, D - n0)
            for p in range(TP):
                xw = work.tile([P, 512], F32, tag="c512b", name="c512b")[:, :nn]
                nc.gpsimd.tensor_tensor(out=xw, in0=src_t[p][:, n0:n0 + nn],
                                        in1=w_b[:, n0:n0 + nn], op=ALU.mult)
                nc.vector.tensor_scalar(out=xw, in0=xw, scalar1=qms[p],
                                        scalar2=MAGIC,
                                        op0=ALU.mult, op1=ALU.add)
                qc = work.tile([P, 512], BF16, tag="qc", name="qc")[:, :nn]
                nc.vector.tensor_scalar(out=qc, in0=xw, scalar1=-MAGIC,
                                        scalar2=None, op0=ALU.add)
                for bi in range(nn // P):
                    ps = pp.tile([P, P], BF16, tag="t", name="tps")
                    nc.tensor.transpose(ps, qc[:, bi * P:(bi + 1) * P],
                                        ident_bf)
                    dst = dstT[ci * 4 + bi][:, p, :]
                    if bi % 2 == 0:
                        nc.vector.tensor_copy(out=dst, in_=ps)
                    else:
                        nc.scalar.copy(out=dst, in_=ps)


def _proj(nc, wpool, mm, lhsT_fn, w3, kb, n_dim, consume):
    """Unswapped projection: out[t, n] = sum_h lhsT(h,p)^T @ w[h, n].

    lhsT_fn(h, p) -> [128, 128] AP (feature-major activation block).
    w3: [128, kb, n_dim] DRAM view.  consume(p, n0, nn, psum)."""
    for n0 in range(0, n_dim, 512):
        nn = min(512, n_dim - n0)
        ps = [mm.tile([P, 512], F32, tag="acc", name="acc")[:, :nn]
              for p in range(TP)]
        for h0 in range(0, kb, HGRP):
            hg = min(HGRP, kb - h0)
            wt = wpool.tile([P, HGRP, 512], FP8, tag="wt",
                            name="wt")[:, :hg, :nn]
            nc.sync.dma_start(out=wt, in_=w3[:, h0:h0 + hg, n0:n0 + nn])
            for j in range(hg):
                h = h0 + j
                for p in range(TP):
                    nc.tensor.matmul(ps[p], lhsT=lhsT_fn(h, p),
                                     rhs=wt[:, j, :],
                                     start=(h == 0), stop=(h == kb - 1))
        for p in range(TP):
            consume(p, n0, nn, ps[p])


def _emit(nc, tc, aps):
    from contextlib import ExitStack

    isq = 1.0 / np.sqrt(float(HD))

    w3 = {k: aps[k].rearrange("(kb p) n -> p kb n", p=P)
          for k in ("wq", "wk", "wv", "wo", "wg", "wu", "wd")}

    ctx = ExitStack()
    with ctx:
        const = ctx.enter_context(tc.tile_pool(name="const", bufs=1))
        small = ctx.enter_context(tc.tile_pool(name="small", bufs=2))
        work = ctx.enter_context(tc.tile_pool(name="work", bufs=2))
        rows = ctx.enter_context(tc.tile_pool(name="rows", bufs=1))
        rwk = ctx.enter_context(tc.tile_pool(name="rwk", bufs=1))
        wpool = ctx.enter_context(tc.tile_pool(name="wpool", bufs=5))
        dram = ctx.enter_context(tc.tile_pool(name="dram", bufs=1, space="DRAM"))

        # ---------------- constants ----------------
        ws_t = []
        for i in range(5):
            t = const.tile([P, 1], F32, tag=f"wsc{i}", name="wsci")
            _bcast_dma(nc, t, aps["wsc"], i, 1)
            ws_t.append(t)
        wsq_t, wsk_t, wsv_t, wso_t, wsd_t = ws_t

        from concourse.masks import make_identity
        ident = const.tile([P, P], F32, tag="ident", name="ident")
        make_identity(nc, ident)
        ident_bf = const.tile([P, P], BF16, tag="identbf", name="identbf")
        make_identity(nc, ident_bf)
        ones_f = const.tile([P, 2], F32, tag="ones", name="ones")
        nc.vector.memset(ones_f, 1.0)
        ones_r = ones_f[:, 0:1].bitcast(F32R)
        ones_h = const.tile([P, 1], F16, tag="onesh", name="onesh")
        nc.vector.memset(ones_h, 1.0)
        ones_row_f = const.tile([1, P], F32, tag="onesrow", name="onesrow")
        nc.vector.memset(ones_row_f, 1.0)
        ones_row_r = ones_row_f.bitcast(F32R)
        eps_t = const.tile([P, 1], F32, tag="epsc", name="epsc")
        nc.vector.memset(eps_t, EPS)
        eps_row = const.tile([1, 1], F32, tag="epsr", name="epsr")
        nc.vector.memset(eps_row, EPS)

        cosT_sb = const.tile([P, T], F32, tag="cosT", name="cosT")
        sinrT_sb = const.tile([P, T], F32, tag="sinrT", name="sinrT")
        nc.sync.dma_start(out=cosT_sb, in_=aps["cosT"])
        nc.sync.dma_start(out=sinrT_sb, in_=aps["sinrT"])
        wsub_fm = const.tile([P, HB], F32, tag="wsubfm", name="wsubfm")
        nc.sync.dma_start(out=wsub_fm, in_=aps["wsub_fm"])
        wffn_fm = const.tile([P, FB], F32, tag="wffnfm", name="wffnfm")
        nc.sync.dma_start(out=wffn_fm, in_=aps["wffn_fm"])

        sqp_h = [small.tile([P, HB // 4], F32, tag=f"nq_sqph{p}", name="sqph")
                 for p in range(TP)]
        mxp_h = [small.tile([P, HB // 4], F32, tag=f"nq_mxph{p}", name="mxph")
                 for p in range(TP)]
        a_o = [None] * TP

        ag_in = [dram.tile([AGC], F32, name=f"ag_in{g}")
                 for g in range(NKV)]
        ag_out = [dram.tile([NCORES, AGC], F32, name=f"ag_out{g}",
                            addr_space="Shared") for g in range(NKV)]

        hpool = ctx.enter_context(tc.tile_pool(name="hpool", bufs=1))
        wvecp = ctx.enter_context(tc.tile_pool(name="wvecp", bufs=1))
        h_tok = [hpool.tile([P, H], F32, tag=f"h{p}", name="hp")
                 for p in range(TP)]

        with tc.tile_pool(name="opool", bufs=1) as opool:
            # ---------------- input norm + quant ----------------
            nch = HB // 4
            qms, a_q, a_k, a_v = [], [], [], []
            with tc.tile_pool(name="kvq", bufs=1) as kvq:
                qT = kvq.tile([P, NH, TP, P], F32R, tag="qT", name="qT")

                with tc.tile_pool(name="xqp", bufs=1) as xqp, \
                     tc.tile_pool(name="ropec", bufs=1) as ropec, \
                     tc.tile_pool(name="rwork", bufs=2) as rwork:
                    xp_ctx = tc.tile_pool(name="xpool", bufs=1)
                    xpool = xp_ctx.__enter__()
                    x_t = [xpool.tile([P, H], F32, tag=f"x{p}", name="xp")
                           for p in range(TP)]
                    wln_b = xpool.tile([P, H], F32, tag="wvec", name="wvec")
                    _bcast_dma(nc, wln_b, aps["wln"], 0, H)
                    kT_own = xqp.tile([P, NKV, TP, P], F32, tag="kT",
                                      name="kT")
                    v_t = [xqp.tile([P, KV], F32, tag=f"v{p}", name="vp")
                           for p in range(TP)]
                    for p in range(TP):
                        nc.sync.dma_start(out=x_t[p],
                                          in_=aps["x"][p * P:(p + 1) * P, :])
                    for p in range(TP):
                        sqp = small.tile([P, nch], F32, tag="nq_sqp",
                                         name="nq_sqp")
                        mxp = small.tile([P, nch], F32, tag="nq_mxp",
                                         name="nq_mxp")
                        for ci in range(nch):
                            n0 = ci * 512
                            _nq_stats_chunk(nc, work, x_t[p][:, n0:n0 + 512],
                                            wln_b[:, n0:n0 + 512],
                                            sqp[:, ci:ci + 1],
                                            mxp[:, ci:ci + 1],
                                            use_gps=(ci % 2 == 1))
                        qm, al = _nq_finalize(nc, small, sqp, mxp,
                                              [(wsq_t, isq / 127.0),
                                               (wsk_t, 1.0 / 127.0),
                                               (wsv_t, 1.0 / 127.0)],
                                              eps_t, H)
                        qms.append(qm)
                        a_q.append(al[0]); a_k.append(al[1]); a_v.append(al[2])
                    xqT = [xqp.tile([P, TP, P], BF16, tag=f"xq{b}", name="xqb")
                           for b in range(HB)]
                    _nq_quant_tp(nc, tc, work, x_t, wln_b, qms, xqT, ident_bf,
                                 "xq")
                    xp_ctx.__exit__(None, None, None)

                    # rope tables pre-scaled by per-token alphas
                    psB_ctx = tc.tile_pool(name="psB", bufs=2, space="PSUM")
                    psB = psB_ctx.__enter__()
                    arow = {}
                    for nm, acols in (("q", a_q), ("k", a_k)):
                        row = rwk.tile([1, T], F32R, tag=f"arow{nm}",
                                         name="arow")
                        for p in range(TP):
                            pst = psB.tile([1, P], F32, tag="tr", name="tr")
                            nc.tensor.transpose(pst, acols[p], ident)
                            nc.vector.tensor_copy(
                                out=row[:, p * P:(p + 1) * P], in_=pst)
                        arow[nm] = row
                    cos_s, sinr_s = {}, {}
                    for nm in ("q", "k"):
                        ps = psB.tile([P, T], F32, tag="ab", name="ab_ps")
                        nc.tensor.matmul(ps, lhsT=ones_row_r, rhs=arow[nm],
                                         start=True, stop=True)
                        ab = ropec.tile([P, T], F32, tag="ab", name="abt")
                        nc.vector.tensor_copy(out=ab, in_=ps)
                        c = ropec.tile([P, T], F32, tag=f"cs{nm}", name="cst")
                        nc.vector.tensor_tensor(out=c, in0=cosT_sb, in1=ab,
                                                op=ALU.mult)
                        sn = ropec.tile([P, T], F32, tag=f"sn{nm}", name="snt")
                        nc.vector.tensor_tensor(out=sn, in0=sinrT_sb, in1=ab,
                                                op=ALU.mult)
                        cos_s[nm], sinr_s[nm] = c, sn
                    psB_ctx.__exit__(None, None, None)

                    def rope_head(ps, dst, nm):
                        """ps: [P, T] psum feature-major head -> dst [P,TP,P]."""
                        ks = rwork.tile([P, T], F32, tag="ks", name="ks")
                        nc.vector.tensor_copy(out=ks, in_=ps)
                        krot = rwork.tile([P, T], F32, tag="krot", name="krot")
                        nc.scalar.dma_start(out=krot[0:H64, :],
                                            in_=ks[H64:P, :])
                        nc.scalar.dma_start(out=krot[H64:P, :],
                                            in_=ks[0:H64, :])
                        nc.vector.tensor_tensor(out=ks, in0=ks,
                                                in1=cos_s[nm], op=ALU.mult)
                        nc.vector.tensor_tensor(out=krot, in0=krot,
                                                in1=sinr_s[nm], op=ALU.mult)
                        nc.vector.tensor_tensor(
                            out=dst.rearrange("d p t -> d (p t)"),
                            in0=ks, in1=krot, op=ALU.add)

                    # ---- K projection (swapped) + per-head AG K parts ----
                    with tc.tile_pool(name="wkp", bufs=2) as wkp, \
                         tc.tile_pool(name="mmKQ", bufs=4, space="PSUM") as mmKQ:
                        for g in range(NKV):
                            wkt = wkp.tile([P, HB, P], FP8, tag="wkt",
                                           name="wkt")
                            nc.sync.dma_start(
                                out=wkt,
                                in_=w3["wk"][:, :, g * P:(g + 1) * P])
                            ps = mmKQ.tile([P, T], F32, tag="kq", name="kq_ps")
                            for h in range(HB):
                                nc.tensor.matmul(
                                    ps, lhsT=wkt[:, h, :],
                                    rhs=xqT[h].rearrange("d p t -> d (p t)"),
                                    start=(h == 0), stop=(h == HB - 1))
                            rope_head(ps, kT_own[:, g], "k")
                            nc.gpsimd.dma_start(
                                out=ag_in[g][0:HD * T]
                                    .rearrange("(d t) -> d t", d=P),
                                in_=kT_own[:, g].rearrange("d p t -> d (p t)"))

                        # ---- V projection (unswapped, token-major) ----
                        with tc.tile_pool(name="mmV", bufs=4,
                                          space="PSUM") as mmV:
                            def eat_v(p, n0, nn, ps):
                                nc.vector.tensor_scalar(
                                    out=v_t[p][:, n0:n0 + nn], in0=ps,
                                    scalar1=a_v[p], scalar2=None, op0=ALU.mult)
                            _proj(nc, wpool, mmV,
                                  lambda h, p: xqT[h][:, p, :], w3["wv"],
                                  HB, KV, eat_v)
                        for g in range(NKV):
                            for p in range(TP):
                                nc.gpsimd.dma_start(
                                    out=ag_in[g][HD * T + p * (P * HD):
                                                 HD * T + (p + 1) * (P * HD)]
                                        .rearrange("(t d) -> t d", t=P),
                                    in_=v_t[p][:, g * P:(g + 1) * P])
                            nc.gpsimd.collective_compute(
                                "AllGather", ALU.bypass,
                                replica_groups=[list(range(NCORES))],
                                ins=[ag_in[g].opt()], outs=[ag_out[g].opt()])

                        # ---- Q projection (swapped; overlaps the AGs) ----
                        with tc.tile_pool(name="wqp", bufs=2) as wqp:
                            for c2 in range(10):
                                wqt = wqp.tile([P, HB, 256], FP8, tag="wqt",
                                               name="wqt")
                                nc.sync.dma_start(
                                    out=wqt,
                                    in_=w3["wq"][:, :, c2 * 256:(c2 + 1) * 256])
                                for i2 in range(2):
                                    hd = c2 * 2 + i2
                                    ps = mmKQ.tile([P, T], F32, tag="kq",
                                                   name="kq_ps")
                                    for h in range(HB):
                                        nc.tensor.matmul(
                                            ps,
                                            lhsT=wqt[:, h, i2 * P:(i2 + 1) * P],
                                            rhs=xqT[h].rearrange(
                                                "d p t -> d (p t)"),
                                            start=(h == 0), stop=(h == HB - 1))
                                    rope_head(ps, qT[:, hd], "q")

                # ---------------- attention ----------------
                o_un, oq = {}, {}
                for g in range(NKV):
                    for p in range(TP):
                        o_un[(g, p)] = opool.tile([P, GQ * P], F32,
                                                  tag=f"ou{g}{p}", name="ou")
                        oq[(g, p)] = opool.tile([P, GQ * P], BF16,
                                                tag=f"oq{g}{p}", name="oqt")
                macc_p = [opool.tile([P, GQ * P], F32, tag=f"macc{p}",
                                     name="macc") for p in range(TP)]
                ssq_sb = [opool.tile([1, GQ * P], F32, tag=f"ssqs{p}",
                                     name="ssqs") for p in range(TP)]
                qm_b = [opool.tile([P, P], F32, tag=f"qmb{p}", name="qmbt")
                        for p in range(TP)]

                with tc.tile_pool(name="attsb", bufs=2) as attp, \
                     tc.tile_pool(name="maskp", bufs=1) as maskp, \
                     tc.tile_pool(name="ptp", bufs=4) as ptp, \
                     tc.tile_pool(name="att2", bufs=2) as att2, \
                     tc.tile_pool(name="awork", bufs=1) as awork, \
                     tc.tile_pool(name="psS", bufs=2, space="PSUM") as psS, \
                     tc.tile_pool(name="psA", bufs=2, space="PSUM") as psA, \
                     tc.tile_pool(name="psD", bufs=2, space="PSUM") as psD, \
                     tc.tile_pool(name="psQ", bufs=2, space="PSUM") as psQ:
                    mask_sb = maskp.tile([P, NCORES, P], BF16, tag="mask",
                                        name="mask")
                    nc.sync.dma_start(out=mask_sb,
                                      in_=aps["mask"].rearrange("r k q -> k r q"))
                    sq_ps = [psQ.tile([1, GQ * P], F32, tag="sq", name="sq_ps")
                             for p in range(TP)]
                    for g in range(NKV):
                        K_g = attp.tile([P, NCORES, TP, P], F32, tag="K",
                                        name="Kg")
                        V_g = attp.tile([P, NCORES, TP, P], F32, tag="V",
                                        name="Vg")
                        for r in range(NCORES):
                            nc.scalar.dma_start(
                                out=K_g[:, r].rearrange("d p t -> d (p t)"),
                                in_=ag_out[g][r, 0:HD * T]
                                    .rearrange("(d t) -> d t", d=P))
                            nc.scalar.dma_start(
                                out=V_g[:, r],
                                in_=ag_out[g][r, HD * T:]
                                    .rearrange("(p t d) -> t p d",
                                               p=TP, t=P))
                        for p in range(TP):
                            ps_att = psA.tile([P, GQ * P], F32, tag="att",
                                              name="att")
                            ps_den = psD.tile([1, GQ * P], F32, tag="den",
                                              name="den")
                            nk = NCORES * (p + 1)
                            idx = 0
                            for h in range(p + 1):
                                for r in range(NCORES):
                                    ps_s = psS.tile([P, GQ * P], F32, tag="s",
                                                    name="s")
                                    nc.tensor.matmul(
                                        ps_s,
                                        lhsT=K_g[:, r, h, :].bitcast(F32R),
                                        rhs=qT[:, GQ * g:GQ * (g + 1), p, :],
                                        start=True, stop=True)
                                    if h == p:
                                        v3 = ps_s.rearrange(
                                            "a (i q) -> a i q", i=GQ)
                                        nc.vector.tensor_tensor(
                                            out=v3, in0=v3,
                                            in1=mask_sb[:, r, None, :]
                                                .to_broadcast((P, GQ, P)),
                                            op=ALU.add)
                                    pt = ptp.tile([P, GQ * P], F32R, tag="pt",
                                                  name="pt")
                                    nc.scalar.activation(out=pt, in_=ps_s,
                                                         func=AF.Exp)
                                    nc.tensor.matmul(
                                        ps_att,
                                        lhsT=V_g[:, r, h, :].bitcast(F32R),
                                        rhs=pt,
                                        start=(idx == 0),
                                        stop=(idx == nk - 1))
                                    nc.tensor.matmul(
                                        ps_den, lhsT=ones_r,
                                        rhs=pt,
                                        start=(idx == 0), stop=(idx == nk - 1))
                                    idx += 1
                            # normalize feature-major: o_n = ps_att / den
                            den_r = att2.tile([1, GQ * P], F32, tag="denr",
                                              name="denr")
                            nc.vector.tensor_copy(out=den_r, in_=ps_den)
                            rden = att2.tile([1, GQ * P], F32R, tag="rden",
                                             name="rden")
                            with nc.allow_low_precision(
                                    reason="f32r is bit-identical to f32"):
                                nc.vector.reciprocal(out=rden, in_=den_r)
                            ps_b = psS.tile([P, GQ * P], F32, tag="s",
                                            name="rdb")
                            nc.tensor.matmul(ps_b, lhsT=ones_row_r, rhs=rden,
                                             start=True, stop=True)
                            rden_b = awork.tile([P, GQ * P], F32, tag="rdenb",
                                                name="rdenb")
                            nc.vector.tensor_copy(out=rden_b, in_=ps_b)
                            ou = o_un[(g, p)]
                            nc.vector.tensor_tensor(out=ou, in0=ps_att,
                                                    in1=rden_b, op=ALU.mult)
                            # stats: sum-of-squares via ones-matmul (PSUM
                            # accumulated across g); absmax via gpsimd
                            sq = awork.tile([P, GQ * P], F32R, tag="sqw",
                                           name="sqw")
                            nc.scalar.activation(out=sq, in_=ou,
                                                 func=AF.Square)
                            nc.tensor.matmul(sq_ps[p], lhsT=ones_r,
                                             rhs=sq,
                                             start=(g == 0),
                                             stop=(g == NKV - 1))
                            mw = awork.tile([P, GQ * P], F32, tag="mww",
                                           name="mww")
                            nc.vector.tensor_tensor(
                                out=mw.rearrange("d (i q) -> d i q", i=GQ),
                                in0=ou.rearrange("d (i q) -> d i q", i=GQ),
                                in1=wsub_fm[:, GQ * g:GQ * (g + 1), None]
                                    .to_broadcast((P, GQ, P)),
                                op=ALU.mult)
                            mw2 = awork.tile([P, GQ * P], F32, tag="mw2",
                                            name="mw2")
                            nc.scalar.activation(out=mw2, in_=mw,
                                                 func=AF.Square)
                            if g == 0:
                                nc.vector.tensor_copy(out=macc_p[p], in_=mw2)
                            else:
                                nc.vector.tensor_tensor(out=macc_p[p],
                                                        in0=macc_p[p],
                                                        in1=mw2, op=ALU.max)

                    # spill the PSUM sum-of-squares rows before pools close
                    for p in range(TP):
                        nc.vector.tensor_copy(out=ssq_sb[p], in_=sq_ps[p])

            # ---- residual reload + o finalize + feature-major quant ----
            x2p_ctx = tc.tile_pool(name="x2pool", bufs=1)
            x2pool = x2p_ctx.__enter__()
            x2_t = [x2pool.tile([P, H], F32, tag=f"x2{p}", name="x2p")
                    for p in range(TP)]
            for p in range(TP):
                nc.sync.dma_start(out=x2_t[p],
                                  in_=aps["x"][p * P:(p + 1) * P, :])
            with tc.tile_pool(name="fwork", bufs=2) as fwork, \
                 tc.tile_pool(name="psF", bufs=2, space="PSUM") as psF:
                    # ---------- o finalize + feature-major quant ----------
                    qm_rows = []
                    for p in range(TP):
                        # absmax: PE-transpose macc blocks + free-axis max
                        mxi = fwork.tile([P, GQ], F32, tag="mxi", name="mxi")
                        for i in range(GQ):
                            ps_i = psF.tile([P, GQ * P], F32, tag="s",
                                            name="tps")[:, :P]
                            nc.tensor.transpose(
                                ps_i, macc_p[p][:, i * P:(i + 1) * P], ident)
                            nc.vector.tensor_reduce(
                                out=mxi[:, i:i + 1], in_=ps_i,
                                axis=mybir.AxisListType.X, op=ALU.max)
                        mx2_col = fwork.tile([P, 1], F32, tag="mx2c",
                                             name="mx2c")
                        nc.vector.tensor_reduce(
                            out=mx2_col, in_=mxi,
                            axis=mybir.AxisListType.X, op=ALU.max)
                        mx_col = fwork.tile([P, 1], F32, tag="mxc",
                                            name="mxc")
                        nc.scalar.activation(out=mx_col, in_=mx2_col,
                                             func=AF.Sqrt)
                        # ssq: i-fold the row then transpose to column
                        ssq_row = fwork.tile([1, P], F32, tag="ssqr",
                                             name="ssqr")
                        nc.vector.tensor_reduce(
                            out=ssq_row,
                            in_=ssq_sb[p].rearrange("a (i q) -> a q i", i=GQ),
                            axis=mybir.AxisListType.X, op=ALU.add)
                        ps_c = psF.tile([P, GQ * P], F32, tag="s",
                                        name="sc")[:, :1]
                        nc.tensor.transpose(ps_c, ssq_row, ident[0:1, 0:1])
                        ssq_col = fwork.tile([P, 1], F32, tag="ssqc",
                                             name="ssqc")
                        nc.vector.tensor_copy(out=ssq_col, in_=ps_c)
                        qm_col, al = _nq_finalize(
                            nc, small, ssq_col, mx_col,
                            [(wso_t, 1.0 / 127.0)], eps_t, H)
                        a_o[p] = al[0]
                        # qm column -> f32r row -> broadcast to [P, P]
                        ps_r = psF.tile([P, GQ * P], F32, tag="s",
                                        name="sr")[0:1, :P]
                        nc.tensor.transpose(ps_r, qm_col, ident)
                        qm_row = rows.tile([1, T], F32R, tag=f"of{p}_qm",
                                           name="r_qm")[:, :P]
                        nc.vector.tensor_copy(out=qm_row, in_=ps_r)
                        ps_b = psF.tile([P, GQ * P], F32, tag="s",
                                        name="qmb")[:, :P]
                        nc.tensor.matmul(ps_b, lhsT=ones_row_r,
                                         rhs=qm_row.bitcast(F32R),
                                         start=True, stop=True)
                        nc.vector.tensor_copy(out=qm_b[p], in_=ps_b)
                    for g in range(NKV):
                        for p in range(TP):
                            u1 = fwork.tile([P, GQ * P], F32, tag="mww",
                                           name="u1")
                            nc.vector.tensor_tensor(
                                out=u1.rearrange("d (i q) -> d i q", i=GQ),
                                in0=o_un[(g, p)].rearrange(
                                    "d (i q) -> d i q", i=GQ),
                                in1=wsub_fm[:, GQ * g:GQ * (g + 1), None]
                                    .to_broadcast((P, GQ, P)),
                                op=ALU.mult)
                            u2 = fwork.tile([P, GQ * P], F32, tag="mw2",
                                           name="u2")
                            nc.vector.tensor_tensor(
                                out=u2.rearrange("d (i q) -> d i q", i=GQ),
                                in0=u1.rearrange("d (i q) -> d i q", i=GQ),
                                in1=qm_b[p][:, None, :]
                                    .to_broadcast((P, GQ, P)),
                                op=ALU.mult)
                            nc.vector.tensor_scalar(out=oq[(g, p)], in0=u2,
                                                    scalar1=MAGIC,
                                                    scalar2=-MAGIC,
                                                    op0=ALU.add, op1=ALU.add)

            # ---------------- o-proj ----------------
            wpost_b = wvecp.tile([P, H], F32, tag="wvec", name="wvec")
            _bcast_dma(nc, wpost_b, aps["wpost"], 0, H)
            with tc.tile_pool(name="mmO", bufs=4, space="PSUM") as mmO:
                def eat_o(p, n0, nn, ps):
                    sl = h_tok[p][:, n0:n0 + nn]
                    nc.vector.tensor_scalar(out=sl, in0=ps, scalar1=a_o[p],
                                            scalar2=None, op0=ALU.mult)
                    nc.vector.tensor_add(out=sl, in0=sl,
                                         in1=x2_t[p][:, n0:n0 + nn])
                    ci = n0 // 512
                    _nq_stats_chunk(nc, work, sl, wpost_b[:, n0:n0 + nn],
                                    sqp_h[p][:, ci:ci + 1],
                                    mxp_h[p][:, ci:ci + 1],
                                    use_gps=(ci % 2 == 1))
                _proj(nc, wpool, mmO,
                      lambda h, p: oq[(h // GQ, p)][:, (h % GQ) * P:
                                                    (h % GQ + 1) * P],
                      w3["wo"], HB, H, eat_o)
            x2p_ctx.__exit__(None, None, None)

        # ---------------- MLP ----------------
        qms_2 = []
        for p in range(TP):
            qm, _ = _nq_finalize(nc, small, sqp_h[p], mxp_h[p], [], eps_t, H)
            qms_2.append(qm)

        with tc.tile_pool(name="xq2p", bufs=1) as xq2p, \
             tc.tile_pool(name="mpool", bufs=1) as mpool:
            xq2T = [xq2p.tile([P, TP, P], BF16, tag=f"x2{b}", name="x2b")
                    for b in range(HB)]
            _nq_quant_tp(nc, tc, work, h_tok, wpost_b, qms_2, xq2T, ident_bf,
                         "xq2")

            m_fm = [mpool.tile([P, T], F32, tag=f"m{b}", name="mb")
                    for b in range(FB)]
            macc2 = mpool.tile([P, T], F32, tag="macc2", name="macc2")

            with tc.tile_pool(name="wgup", bufs=2) as wgup, \
                 tc.tile_pool(name="mwork", bufs=1) as mwork, \
                 tc.tile_pool(name="psG", bufs=4, space="PSUM") as psG, \
                 tc.tile_pool(name="psQ2", bufs=1, space="PSUM") as psQ2:
                sq2_ps = psQ2.tile([1, T], F32, tag="sq2", name="sq2")
                for fb4 in range(0, FB, 2):
                    nfb = min(2, FB - fb4)
                    wtg = wgup.tile([P, HB, 256], FP8, tag="wtg",
                                    name="wtg")[:, :, :nfb * P]
                    wtu = wgup.tile([P, HB, 256], FP8, tag="wtu",
                                    name="wtu")[:, :, :nfb * P]
                    nc.sync.dma_start(
                        out=wtg, in_=w3["wg"][:, :, fb4 * P:(fb4 + nfb) * P])
                    nc.sync.dma_start(
                        out=wtu, in_=w3["wu"][:, :, fb4 * P:(fb4 + nfb) * P])
                    for j in range(nfb):
                        fb = fb4 + j
                        ps_g = psG.tile([P, T], F32, tag="gu", name="gps")
                        ps_u = psG.tile([P, T], F32, tag="gu", name="ups")
                        for h in range(HB):
                            rhs = xq2T[h].rearrange("d p t -> d (p t)")
                            nc.tensor.matmul(
                                ps_g, lhsT=wtg[:, h, j * P:(j + 1) * P],
                                rhs=rhs, start=(h == 0), stop=(h == HB - 1))
                            nc.tensor.matmul(
                                ps_u, lhsT=wtu[:, h, j * P:(j + 1) * P],
                                rhs=rhs, start=(h == 0), stop=(h == HB - 1))
                        gr = mwork.tile([P, T], F32, tag="gr", name="gr")
                        nc.vector.tensor_scalar(out=gr, in0=ps_g, scalar1=0.0,
                                                scalar2=None, op0=ALU.max)
                        gr2 = mwork.tile([P, T], F32, tag="gr2", name="gr2")
                        nc.scalar.activation(out=gr2, in_=gr, func=AF.Square)
                        mf = m_fm[fb]
                        nc.vector.tensor_tensor(out=mf, in0=gr2, in1=ps_u,
                                                op=ALU.mult)
                        mw = mwork.tile([P, T], F32, tag="mwf", name="mwf")
                        nc.vector.tensor_scalar(
                            out=mw, in0=mf, scalar1=wffn_fm[:, fb:fb + 1],
                            scalar2=None, op0=ALU.mult)
                        mw2 = mwork.tile([P, T], F32, tag="mw2f", name="mw2f")
                        nc.scalar.activation(out=mw2, in_=mw, func=AF.Square)
                        if fb == 0:
                            nc.vector.tensor_copy(out=macc2, in_=mw2)
                        else:
                            nc.vector.tensor_tensor(out=macc2, in0=macc2,
                                                    in1=mw2, op=ALU.max)
                        m2 = mwork.tile([P, T], F32R, tag="m2f", name="m2f")
                        nc.scalar.activation(out=m2, in_=mf, func=AF.Square)
                        nc.tensor.matmul(sq2_ps, lhsT=ones_r,
                                         rhs=m2,
                                         start=(fb == 0), stop=(fb == FB - 1))

                # m finalize (transpose-based, column form)
                ssq2_row = rwk.tile([1, T], F32, tag="ssq2", name="ssq2")
                nc.vector.tensor_copy(out=ssq2_row, in_=sq2_ps)
                qm2_row = rows.tile([1, T], F32R, tag="mf_qm", name="r_qm")
                a_d = []
                with tc.tile_pool(name="psM", bufs=2, space="PSUM") as psM:
                    for p in range(TP):
                        ps_i = psM.tile([P, T], F32, tag="b",
                                        name="tps")[:, :P]
                        nc.tensor.transpose(
                            ps_i, macc2[:, p * P:(p + 1) * P], ident)
                        mx2c = small.tile([P, 1], F32, tag="m_mx2c",
                                          name="mx2c")
                        nc.vector.tensor_reduce(
                            out=mx2c, in_=ps_i,
                            axis=mybir.AxisListType.X, op=ALU.max)
                        mxc = small.tile([P, 1], F32, tag="m_mxc",
                                         name="mxc")
                        nc.scalar.activation(out=mxc, in_=mx2c, func=AF.Sqrt)
                        ps_c = psM.tile([P, T], F32, tag="b",
                                        name="sc")[:, :1]
                        nc.tensor.transpose(
                            ps_c, ssq2_row[:, p * P:(p + 1) * P],
                            ident[0:1, 0:1])
                        ssq_col = small.tile([P, 1], F32, tag="m_ssqc",
                                             name="ssqc")
                        nc.vector.tensor_copy(out=ssq_col, in_=ps_c)
                        qm2_col, al = _nq_finalize(
                            nc, small, ssq_col, mxc,
                            [(wsd_t, 1.0 / 127.0)], eps_t, FF)
                        a_d.append(al[0])
                        ps_r = psM.tile([P, T], F32, tag="b",
                                        name="sr")[0:1, :P]
                        nc.tensor.transpose(ps_r, qm2_col, ident)
                        nc.vector.tensor_copy(
                            out=qm2_row[:, p * P:(p + 1) * P], in_=ps_r)
                    ps_b = psM.tile([P, T], F32, tag="b", name="qm2b")
                    nc.tensor.matmul(ps_b, lhsT=ones_row_r,
                                     rhs=qm2_row.bitcast(F32R),
                                     start=True, stop=True)
                    qm2_b = mpool.tile([P, T], F32, tag="qm2b", name="qm2bt")
                    nc.vector.tensor_copy(out=qm2_b, in_=ps_b)

            # m quantize (feature-major, no transposes)
            mq = [mpool.tile([P, T], BF16, tag=f"mq{b}", name="mqb")
                  for b in range(FB)]
            with tc.tile_pool(name="qwork", bufs=1) as qwork:
                for fb in range(FB):
                    u1 = qwork.tile([P, T], F32, tag="mwf", name="u1m")
                    nc.vector.tensor_scalar(out=u1, in0=m_fm[fb],
                                            scalar1=wffn_fm[:, fb:fb + 1],
                                            scalar2=None, op0=ALU.mult)
                    u2 = qwork.tile([P, T], F32, tag="mw2f", name="u2m")
                    nc.vector.tensor_tensor(out=u2, in0=u1, in1=qm2_b,
                                            op=ALU.mult)
                    nc.vector.tensor_scalar(out=mq[fb], in0=u2, scalar1=MAGIC,
                                            scalar2=-MAGIC, op0=ALU.add,
                                            op1=ALU.add)

            # down-proj
            with tc.tile_pool(name="mmF", bufs=4, space="PSUM") as mmF:
                def eat_d(p, n0, nn, ps):
                    o_sb = work.tile([P, 512], F32, tag="c512a",
                                     name="osb")[:, :nn]
                    nc.vector.tensor_scalar(out=o_sb, in0=ps, scalar1=a_d[p],
                                            scalar2=None, op0=ALU.mult)
                    nc.vector.tensor_add(out=o_sb, in0=o_sb,
                                         in1=h_tok[p][:, n0:n0 + nn])
                    nc.sync.dma_start(out=aps["out"][p * P:(p + 1) * P,
                                                     n0:n0 + nn],
                                      in_=o_sb)
                _proj(nc, wpool, mmF,
                      lambda h, p: mq[h][:, p * P:(p + 1) * P],
                      w3["wd"], FB, H, eat_d)


_NC_CACHE = {}


def _get_nc():
    if "nc" not in _NC_CACHE:
        _NC_CACHE["nc"] = _build_nc()
    return _NC_CACHE["nc"]


def _quant_w(w):
    w = np.asarray(w, np.float32)
    ws = np.maximum(np.float32(np.abs(w).mean(dtype=np.float32)),
                    np.float32(1e-5))
    wq = np.clip(np.round(w / ws), -1.0, 1.0).astype(np.float32)
    return wq, float(ws)


def kernel(hidden_states, cos, sin, w_in_ln, w_q, w_k, w_v, w_o,
           w_attn_sub, w_post_ln, w_gate, w_up, w_ffn_sub, w_down,
           _trace=False):
    hs = np.asarray(hidden_states, np.float32)
    assert hs.shape == (1, S, H)

    nc = _get_nc()

    wq_i, s_q = _quant_w(w_q)
    wk_i, s_k = _quant_w(w_k)
    wv_i, s_v = _quant_w(w_v)
    wo_i, s_o = _quant_w(w_o)
    wg_i, _ = _quant_w(w_gate)
    wu_i, _ = _quant_w(w_up)
    wd_i, s_d = _quant_w(w_down)

    f8 = ml_dtypes.float8_e4m3
    shared = {
        "wq": np.ascontiguousarray(wq_i.T).astype(f8),
        "wk": np.ascontiguousarray(wk_i.T).astype(f8),
        "wv": np.ascontiguousarray(wv_i.T).astype(f8),
        "wo": np.ascontiguousarray(wo_i.T).astype(f8),
        "wg": np.ascontiguousarray(wg_i.T).astype(f8),
        "wu": np.ascontiguousarray(wu_i.T).astype(f8),
        "wd": np.ascontiguousarray(wd_i.T).astype(f8),
        "wln": np.asarray(w_in_ln, np.float32),
        "wsub_fm": np.ascontiguousarray(
            np.asarray(w_attn_sub, np.float32).reshape(HB, P).T),
        "wpost": np.asarray(w_post_ln, np.float32),
        "wffn_fm": np.ascontiguousarray(
            np.asarray(w_ffn_sub, np.float32).reshape(FB, P).T),
        "wsc": np.array([s_q, s_k, s_v, s_o, s_d], np.float32),
    }

    cos0 = np.asarray(cos, np.float32)[0]    # [S, HD]
    sin0 = np.asarray(sin, np.float32)[0]
    sinr = sin0.copy()
    sinr[:, :HD // 2] = -sin0[:, :HD // 2]

    x_resh = hs[0].reshape(T, NCORES, H)
    cos_resh = cos0.reshape(T, NCORES, HD)
    sinr_resh = sinr.reshape(T, NCORES, HD)

    kk, qq = np.meshgrid(np.arange(P), np.arange(P), indexing="ij")
    in_maps = []
    for c in range(NCORES):
        masks = np.empty((NCORES, P, P), np.float32)
        for r in range(NCORES):
            lim = qq - (1 if r > c else 0)
            masks[r] = np.where(kk <= lim, 0.0, NEG)
        m = dict(shared)
        m["x"] = np.ascontiguousarray(x_resh[:, c, :])
        m["cosT"] = np.ascontiguousarray(cos_resh[:, c, :].T)
        m["sinrT"] = np.ascontiguousarray(sinr_resh[:, c, :].T)
        m["mask"] = masks.astype(ml_dtypes.bfloat16)
        in_maps.append(m)

    res = bass_utils.run_bass_kernel_spmd(
        nc, in_maps, core_ids=list(range(NCORES)), trace=_trace)

    out = np.empty((1, S, H), np.float32)
    out_resh = out[0].reshape(T, NCORES, H)
    for c in range(NCORES):
        out_resh[:, c, :] = res.results[c]["out"]

    kernel._last_results = res
    return out
